# revision 8
# baseline (speedup 1.0000x reference)
"""Bass/Trainium2 kernel for nn_F_Loss_65446711656630.

Strategy (data-parallel over N, 8 cores):
  - Host: per core, sort the core's 8192 rows by class id and transpose to
    [512 features, 8192 rows] (contiguous).  After sorting, almost every
    128-row chunk is single-class.
  - Device (static kernel, no data-dependent structure): stream [128, 1024]
    pieces; per piece compute per-128-row-chunk partial sums (DVE
    multi-chunk reduce via 3D access pattern) and partial sums-of-squares
    (ACT square + DVE reduce).  Pure f32, no matmul, PE idle; this is the
    memory-bound part (128 MiB streamed at HBM rate).
  - Host: per-class stats = sum of single-class chunk partials (fp64)
    + direct numpy sums for the few class-boundary chunks; then the tiny
    O(C^2 D) pairwise betainc/top-k stage (C=16) on CPU.
"""

import numpy as np

C = 16
D = 512
N = 65536
NCORES = 8
ROWS = N // NCORES          # 8192 rows per core
P = 128                     # SBUF partitions
PIECE = 1024                # rows per DMA piece
X = 128                     # rows per reduction chunk
NBLK = D // P               # 4 feature blocks
NPIECE = ROWS // PIECE      # 8 pieces per block
NCHUNK = ROWS // X          # 64 chunks per core
CPP = PIECE // X            # 8 chunks per piece
XMIN, XMAX = 1e-37, 1.0 - 1e-5

_NC_CACHE = {}


def _build_nc():
    """Per-core SPMD program: chunkwise partial sums / sums-of-squares.

    Inputs:  "ht"   [512, 8192]  f32  (sorted, transposed hidden)
    Outputs: "hsum" [128, 256]   f32  (hsum[f, b*64+g] = sum over chunk g's
                                       rows of feature b*128+f)
             "ssum" [128, 256]   f32  (same for squares)
    """
    import concourse.tile as tile
    from concourse import bacc, mybir

    f32 = mybir.dt.float32

    nc = bacc.Bacc("TRN2", target_bir_lowering=False, debug=False,
                   num_devices=NCORES)
    ht = nc.declare_dram_parameter("ht", [D, ROWS], f32, isOutput=False)
    hsum = nc.declare_dram_parameter("hsum", [P, NBLK * NCHUNK], f32, isOutput=True)
    ssum = nc.declare_dram_parameter("ssum", [P, NBLK * NCHUNK], f32, isOutput=True)

    ht_v = ht[:].rearrange("(b p) r -> b p r", p=P)  # [4, 128, 8192]

    with tile.TileContext(nc) as tc:
        with (
            tc.tile_pool(name="pc", bufs=4) as piece_pool,
            tc.tile_pool(name="sq", bufs=4) as sq_pool,
            tc.tile_pool(name="acc", bufs=1) as acc_pool,
        ):
            hpart = acc_pool.tile([P, NBLK * NCHUNK], f32, tag="hpart")
            spart = acc_pool.tile([P, NBLK * NCHUNK], f32, tag="spart")

            for b in range(NBLK):
                for p in range(NPIECE):
                    t = piece_pool.tile([P, PIECE], f32)
                    nc.sync.dma_start(t[:], ht_v[b, :, p * PIECE:(p + 1) * PIECE])
                    sq = sq_pool.tile([P, PIECE], f32)
                    nc.scalar.square(sq[:], t[:])

                    base = b * NCHUNK + p * CPP
                    t3 = t[:].rearrange("p (c x) -> p c x", x=X)
                    s3 = sq[:].rearrange("p (c x) -> p c x", x=X)
                    nc.vector.reduce_sum(
                        hpart[:, base:base + CPP], t3, axis=mybir.AxisListType.X)
                    nc.vector.reduce_sum(
                        spart[:, base:base + CPP], s3, axis=mybir.AxisListType.X)

            nc.sync.dma_start(hsum[:], hpart[:])
            nc.sync.dma_start(ssum[:], spart[:])
    nc.compile()
    return nc


def _get_nc():
    if "nc" not in _NC_CACHE:
        _NC_CACHE["nc"] = _build_nc()
    return _NC_CACHE["nc"]


def _prep_core(hidden_k, ids_k):
    """Sort rows by class, transpose; classify chunks; boundary-row stats."""
    order = np.argsort(ids_k, kind="stable")
    ids_sorted = ids_k[order]
    hs = hidden_k[order]                         # [8192, 512] f32, sorted
    T = np.ascontiguousarray(hs.T)               # [512, 8192] f32

    cm = ids_sorted.reshape(NCHUNK, X)
    interior = cm[:, 0] == cm[:, -1]             # single-class chunk?
    chunk_class = np.where(interior, cm[:, 0], -1).astype(np.int64)

    bsum = np.zeros((C, D), dtype=np.float64)
    bsq = np.zeros((C, D), dtype=np.float64)
    if not interior.all():
        bmask = np.repeat(~interior, X)
        brows = hs[bmask].astype(np.float64)     # rows in boundary chunks
        bids = ids_sorted[bmask]
        for q in np.unique(bids):
            sel = brows[bids == q]
            bsum[q] = sel.sum(axis=0)
            bsq[q] = (sel * sel).sum(axis=0)
    return T, chunk_class, bsum, bsq


def _device_stats(hidden, ids, **run_kwargs):
    """Returns (sums[C,D], sumsq[C,D]) float64, plus the raw run result."""
    from concourse import bass_utils

    nc = _get_nc()
    in_maps = []
    chunk_classes = []
    sums = np.zeros((C, D), dtype=np.float64)
    sumsq = np.zeros((C, D), dtype=np.float64)
    for k in range(NCORES):
        rows = slice(k * ROWS, (k + 1) * ROWS)
        T, cls, bsum, bsq = _prep_core(hidden[rows], ids[rows])
        in_maps.append({"ht": T})
        chunk_classes.append(cls)
        sums += bsum
        sumsq += bsq

    res = bass_utils.run_bass_kernel_spmd(nc, in_maps, list(range(NCORES)), **run_kwargs)

    for k in range(NCORES):
        # dev output [128, 4*64]: col b*64+g, row f -> feature b*128+f, chunk g
        hp = res.results[k]["hsum"].astype(np.float64)
        sp = res.results[k]["ssum"].astype(np.float64)
        # -> [chunk, feature]
        hp = hp.reshape(P, NBLK, NCHUNK).transpose(2, 1, 0).reshape(NCHUNK, D)
        sp = sp.reshape(P, NBLK, NCHUNK).transpose(2, 1, 0).reshape(NCHUNK, D)
        cls = chunk_classes[k]
        sel = cls >= 0
        oh = (cls[sel, None] == np.arange(C)[None, :]).astype(np.float64)  # [g, C]
        sums += oh.T @ hp[sel]
        sumsq += oh.T @ sp[sel]
    return sums, sumsq, res


def _pairwise_loss(counts, sums, sumsq, d):
    """The tiny O(C^2 D) stage, float64 on host. Mirrors reference.py exactly."""
    from scipy.special import betainc as sp_betainc

    counts = counts.astype(np.float64)
    means = sums / counts[:, None]                                # [C, D]
    withins = sumsq - counts[:, None] * means**2                  # [C, D]
    half_diff = (means[:, None, :] - means[None, :, :]) * 0.5     # [C, C, D]
    pair_counts = counts[:, None] + counts[None, :]               # [C, C]
    pair_between = half_diff * half_diff * pair_counts[:, :, None]
    pair_within = withins[:, None, :] + withins[None, :, :]
    d2 = pair_counts - 2.0
    d2 = np.where(d2 == 0.0, 1e-5, d2)
    with np.errstate(invalid="ignore", divide="ignore"):
        x = pair_between / (pair_between + pair_within)
    x = np.clip(x, XMIN, XMAX)
    b = np.broadcast_to((d2 * 0.5)[:, :, None], x.shape)
    xbetainc = sp_betainc(0.5, b, x)                              # [C, C, D]
    k = int(d)
    top_k = np.partition(xbetainc, D - k, axis=-1)[..., D - k:]   # [C, C, d]
    per_pair = np.sum(np.log(top_k), axis=-1)                     # [C, C]
    mask = np.triu(np.ones((C, C), dtype=bool), k=1)
    total = np.sum(np.where(mask, per_pair, 0.0))
    return -total


def kernel(hidden, batch_ids, d):
    hidden = np.asarray(hidden, dtype=np.float32)
    ids = np.asarray(batch_ids).astype(np.int64)
    assert hidden.shape == (N, D), hidden.shape

    counts = np.bincount(ids, minlength=C).astype(np.float64)
    sums, sumsq, _ = _device_stats(hidden, ids)
    total = _pairwise_loss(counts, sums, sumsq, int(np.asarray(d)))
    return np.array(total, dtype=np.float32)
